# revision 20
# baseline (speedup 1.0000x reference)
"""MoE (T=1024, H=1024, F=2048, E=8, top-k=2) on 8 trn2 cores.

Pair-tensor-parallel expert layout: experts are paired (largest bucket
with smallest) and each pair is served by two cores that split the ffn
dim in half. Core (pair p, half h) holds, for both experts of the pair,
the gate/lin rows [h*F/2,(h+1)*F/2) of w1 and the matching w2 columns,
and processes ALL tokens routed to either expert. fc2 contracts only
the local half of F, so the two halves produce additive partial sums
(b2 is halved on each) that the host scatter-add combines.

Per-core matmul cycles drop from 384*Cmax to 192*(C0+C1), where C0/C1
are the per-slot capacities (max larger-expert count / max smaller),
~4-5% less than pure expert parallelism at identical weight traffic
(12MB bf16 per core, no duplication).

All matmuls are bf16 (weights+activations), fp32 PSUM. A burst of
dependency-free warmup matmuls on a memset tile ramps the PE to full
p-state during the initial DMA window.
"""

import numpy as np
from contextlib import ExitStack

import concourse.bass as bass
import concourse.mybir as mybir
import concourse.tile as tile
from concourse import bacc
from concourse.bass_utils import run_bass_kernel_spmd

import ml_dtypes

T, H, F, E, TOPK = 1024, 1024, 2048, 8, 2
P = 128
FH = F // 2        # per-core half of the ffn inter dim
KH = H // P        # 8   fc1 contraction chunks
MGH = FH // P      # 8   gate m-chunks per expert (lin likewise)
KFH = FH // P      # 8   fc2 contraction chunks (local half)
M2 = H // P        # 8   fc2 output chunks
C_ALIGN = 4
C_MAX = 512
F32 = mybir.dt.float32
BF16 = mybir.dt.bfloat16
NP_BF16 = ml_dtypes.bfloat16

TRACE = False
TRACE_KWARGS = {}
LAST_RESULT = None

_nc_cache = {}


def _build_nc(C0: int, C1: int) -> bass.Bass:
    nc = bacc.Bacc("TRN2", target_bir_lowering=False, debug=False)
    CS = (C0, C1)
    xs0 = nc.dram_tensor("xs0", [P, KH, C0], BF16, kind="ExternalInput")
    xs1 = nc.dram_tensor("xs1", [P, KH, C1], BF16, kind="ExternalInput")
    # per expert slot: [jj, s, g, p, k, n] with m = 2*jj+s (8 gate m-chunks)
    w1s = nc.dram_tensor("w1s", [2, MGH // 2, 2, 2, P, KH, P], BF16, kind="ExternalInput")
    # per expert slot: [mm, s, p, k, n] with m = 2*mm+s (8 out m-chunks, k<8)
    w2s = nc.dram_tensor("w2s", [2, M2 // 2, 2, P, KFH, P], BF16, kind="ExternalInput")
    # per slot: 2*MGH gate+lin bias cols then M2 b2/2 cols
    bs = nc.dram_tensor("bs", [2, P, 2 * MGH + M2], F32, kind="ExternalInput")
    ys0 = nc.dram_tensor("ys0", [M2, P, C0], BF16, kind="ExternalOutput")
    ys1 = nc.dram_tensor("ys1", [M2, P, C1], BF16, kind="ExternalOutput")
    YS = (ys0, ys1)

    silu = mybir.ActivationFunctionType.Silu

    with tile.TileContext(nc) as tc, ExitStack() as ctx:
        consts = ctx.enter_context(tc.tile_pool(name="consts", bufs=1))
        xpool = ctx.enter_context(tc.tile_pool(name="xpool", bufs=2))
        w1pool = ctx.enter_context(tc.tile_pool(name="w1pool", bufs=4))
        w2pool = ctx.enter_context(tc.tile_pool(name="w2pool", bufs=3))
        actpool = ctx.enter_context(tc.tile_pool(name="actpool", bufs=2))
        evpool = ctx.enter_context(tc.tile_pool(name="evpool", bufs=4))
        ypool = ctx.enter_context(tc.tile_pool(name="ypool", bufs=3))
        ps1 = ctx.enter_context(tc.tile_pool(name="ps1", bufs=4, space="PSUM"))
        ps2 = ctx.enter_context(tc.tile_pool(name="ps2", bufs=2, space="PSUM"))
        pswarm = ctx.enter_context(tc.tile_pool(name="pswarm", bufs=1, space="PSUM"))

        # PE p-state warmup during the initial DMA window
        warm = consts.tile([P, 256], BF16)
        nc.gpsimd.memset(warm, 0)
        pwarm = pswarm.tile([P, 256], F32)
        for i in range(18):
            nc.tensor.matmul(
                pwarm, lhsT=warm[:, :P], rhs=warm, start=(i == 0), stop=(i == 17)
            )

        # Startup: slot-0 x and the s=1 w1 slice ride SP ahead of the
        # bulk jj stream; the two s=0 w1 slices + bias + slot-1 x ride ACT
        # in parallel.
        x0_sb = xpool.tile([P, KH, C0], BF16, tag="x0")
        nc.sync.dma_start(out=x0_sb[:, : KH // 2], in_=xs0[:, : KH // 2, :])
        w1_first = w1pool.tile([P, 2, 2, KH, P], BF16, tag="w1")
        nc.scalar.dma_start(out=w1_first[:, 0, 0], in_=w1s[0, 0, 0, 0])
        nc.sync.dma_start(out=x0_sb[:, KH // 2 :], in_=xs0[:, KH // 2 :, :])
        nc.scalar.dma_start(out=w1_first[:, 0, 1], in_=w1s[0, 0, 0, 1])
        nc.sync.dma_start(
            out=w1_first[:, 1], in_=w1s[0, 0, 1].rearrange("g p k n -> p g k n")
        )
        b_sb = consts.tile([P, 2, 2 * MGH + M2], F32)
        nc.scalar.dma_start(out=b_sb[:, 0], in_=bs[0])
        nc.scalar.dma_start(out=b_sb[:, 1], in_=bs[1])
        x1_sb = xpool.tile([P, KH, C1], BF16, tag="x1")
        nc.scalar.dma_start(out=x1_sb, in_=xs1[:, :, :])

        XS = (x0_sb, x1_sb)

        for t in range(2):
            C = CS[t]
            x_sb = XS[t]
            b1_sb = b_sb[:, t, : 2 * MGH]
            b2_sb = b_sb[:, t, 2 * MGH :]
            act_all = actpool.tile([P, KFH, C], BF16, tag=f"act{t}")

            for jj in range(MGH // 2):
                if t == 0 and jj == 0:
                    w1_sb = w1_first
                else:
                    w1_sb = w1pool.tile([P, 2, 2, KH, P], BF16, tag="w1")
                    nc.sync.dma_start(
                        out=w1_sb, in_=w1s[t, jj].rearrange("s g p k n -> p s g k n")
                    )
                for s in range(2):
                    m = 2 * jj + s
                    pg = ps1.tile([P, C], F32, tag="ps1")
                    pl = ps1.tile([P, C], F32, tag="ps1")
                    for k in range(KH):
                        nc.tensor.matmul(
                            pg,
                            lhsT=w1_sb[:, s, 0, k, :],
                            rhs=x_sb[:, k, :],
                            start=(k == 0),
                            stop=(k == KH - 1),
                        )
                    for k in range(KH):
                        nc.tensor.matmul(
                            pl,
                            lhsT=w1_sb[:, s, 1, k, :],
                            rhs=x_sb[:, k, :],
                            start=(k == 0),
                            stop=(k == KH - 1),
                        )
                    gate_sb = evpool.tile([P, C], F32, tag="gate")
                    nc.scalar.activation(gate_sb, pg, silu, bias=b1_sb[:, m : m + 1])
                    lin_sb = evpool.tile([P, C], F32, tag="lin")
                    nc.vector.tensor_scalar_add(
                        lin_sb, pl, b1_sb[:, MGH + m : MGH + m + 1]
                    )
                    nc.vector.tensor_mul(act_all[:, m, :], gate_sb, lin_sb)

            ys = YS[t]
            for mm in range(M2 // 2):
                w2_sb = w2pool.tile([P, 2, KFH, P], BF16, tag="w2")
                if t == 1 and mm == M2 // 2 - 1:
                    nc.sync.dma_start(out=w2_sb[:, 0], in_=w2s[t, mm, 0])
                    nc.sync.dma_start(out=w2_sb[:, 1], in_=w2s[t, mm, 1])
                else:
                    nc.sync.dma_start(
                        out=w2_sb, in_=w2s[t, mm].rearrange("s p k n -> p s k n")
                    )
                y_sb = ypool.tile([P, 2, C], BF16, tag="y")
                for s in range(2):
                    m = 2 * mm + s
                    p2 = ps2.tile([P, C], F32, tag="ps2")
                    for k in range(KFH):
                        nc.tensor.matmul(
                            p2,
                            lhsT=w2_sb[:, s, k, :],
                            rhs=act_all[:, k, :],
                            start=(k == 0),
                            stop=(k == KFH - 1),
                        )
                    if t == 1 and mm == M2 // 2 - 1 and s == 1:
                        hc = C // 2
                        nc.vector.tensor_scalar_add(
                            y_sb[:, s, :hc], p2[:, :hc], b2_sb[:, m : m + 1]
                        )
                        nc.scalar.dma_start(out=ys[m][:, :hc], in_=y_sb[:, s, :hc])
                        nc.vector.tensor_scalar_add(
                            y_sb[:, s, hc:], p2[:, hc:], b2_sb[:, m : m + 1]
                        )
                        nc.sync.dma_start(out=ys[m][:, hc:], in_=y_sb[:, s, hc:])
                    else:
                        nc.vector.tensor_scalar_add(
                            y_sb[:, s, :], p2, b2_sb[:, m : m + 1]
                        )
                        nc.scalar.dma_start(out=ys[m], in_=y_sb[:, s, :])

    nc.compile()
    return nc


def _get_nc(C0: int, C1: int) -> bass.Bass:
    key = (C0, C1)
    if key not in _nc_cache:
        _nc_cache[key] = _build_nc(C0, C1)
    return _nc_cache[key]


def _pack_expert_half(w1, b1, w2, b2, e, h):
    """Pack expert e's ffn-half h into device layouts (bf16 weights)."""
    gsl = slice(h * FH, (h + 1) * FH)
    lsl = slice(F + h * FH, F + (h + 1) * FH)
    w1h = np.concatenate([w1[e][gsl], w1[e][lsl]], axis=0)  # [2*FH, H]
    w1c = np.ascontiguousarray(
        w1h.astype(NP_BF16).reshape(2 * MGH, P, KH, P).transpose(0, 3, 2, 1)
    )
    # stack gate/lin m-chunk pairs like the EP kernel: [jj, s, g, p, k, n]
    w1se = np.ascontiguousarray(
        np.stack(
            [
                w1c[:MGH].reshape(MGH // 2, 2, P, KH, P),
                w1c[MGH:].reshape(MGH // 2, 2, P, KH, P),
            ],
            axis=2,
        )
    )
    w2h = w2[e][:, gsl]  # [H, FH]
    w2c = w2h.astype(NP_BF16).reshape(M2, P, KFH, P).transpose(0, 3, 2, 1)
    w2se = np.ascontiguousarray(w2c.reshape(M2 // 2, 2, P, KFH, P))
    b1h = np.concatenate([b1[e][gsl], b1[e][lsl]])  # [2*FH]
    bse = np.ascontiguousarray(
        np.concatenate(
            [b1h.reshape(2 * MGH, P), (0.5 * b2[e]).reshape(M2, P)], 0
        ).T
    )
    return w1se, w2se, bse


def kernel(
    hidden_states,
    token_selected_experts,
    token_final_scales,
    w1,
    b1,
    w2,
    b2,
):
    global LAST_RESULT
    hs = np.ascontiguousarray(np.asarray(hidden_states, dtype=np.float32))
    sel = np.asarray(token_selected_experts, dtype=np.int32)
    scl = np.asarray(token_final_scales, dtype=np.float32)
    w1 = np.asarray(w1, dtype=np.float32)
    b1 = np.asarray(b1, dtype=np.float32)
    w2 = np.asarray(w2, dtype=np.float32)
    b2 = np.asarray(b2, dtype=np.float32)

    nt, hh = hs.shape
    assert (nt, hh) == (T, H), f"unexpected shape {hs.shape}"

    flat_e = sel.reshape(-1)
    slot_tok = np.repeat(np.arange(T, dtype=np.int64), TOPK)
    order = np.argsort(flat_e, kind="stable")
    sorted_tok = slot_tok[order]
    sorted_scl = scl.reshape(-1)[order]
    counts = np.bincount(flat_e, minlength=E)
    starts = np.concatenate([[0], np.cumsum(counts)])

    # pair largest with smallest by bucket size
    rank = np.argsort(counts)[::-1]          # expert ids sorted desc by count
    pairs = [(int(rank[p]), int(rank[E - 1 - p])) for p in range(E // 2)]
    c0_need = max(int(counts[a]) for a, _ in pairs)
    c1_need = max(int(counts[b]) for _, b in pairs)
    C0 = min(C_MAX, -(-max(1, c0_need) // C_ALIGN) * C_ALIGN)
    C1 = min(C_MAX, -(-max(1, c1_need) // C_ALIGN) * C_ALIGN)
    n_chunks = max(1, -(-c0_need // C0), -(-c1_need // C1))

    nc = _get_nc(C0, C1)

    out = np.zeros((T, H), dtype=np.float32)
    for ci in range(n_chunks):
        in_maps = []
        metas = []
        for p, (ea, eb) in enumerate(pairs):
            # gather token batches for both experts of the pair
            xses, idss, scls = [], [], []
            for t, (e, C) in enumerate(((ea, C0), (eb, C1))):
                lo = int(starts[e]) + ci * C
                hi = min(int(starts[e + 1]), lo + C)
                ids = sorted_tok[lo:hi] if hi > lo else np.empty(0, np.int64)
                xg = np.zeros((C, H), dtype=NP_BF16)
                if len(ids):
                    xg[: len(ids)] = hs[ids].astype(NP_BF16)
                xses.append(
                    np.ascontiguousarray(xg.T.reshape(KH, P, C).transpose(1, 0, 2))
                )
                idss.append(ids)
                scls.append(sorted_scl[lo:hi] if len(ids) else None)
            for h in range(2):
                w1a, w2a, ba = _pack_expert_half(w1, b1, w2, b2, ea, h)
                w1b, w2b, bb = _pack_expert_half(w1, b1, w2, b2, eb, h)
                in_maps.append(
                    {
                        "xs0": xses[0],
                        "xs1": xses[1],
                        "w1s": np.ascontiguousarray(np.stack([w1a, w1b])),
                        "w2s": np.ascontiguousarray(np.stack([w2a, w2b])),
                        "bs": np.ascontiguousarray(np.stack([ba, bb])),
                    }
                )
                metas.append((idss, scls))

        res = run_bass_kernel_spmd(
            nc,
            in_maps,
            core_ids=list(range(E)),
            trace=TRACE,
            **TRACE_KWARGS,
        )
        LAST_RESULT = res
        for core in range(E):
            idss, scls = metas[core]
            for t, (ids, ss, C) in enumerate(
                ((idss[0], scls[0], C0), (idss[1], scls[1], C1))
            ):
                if ids is None or len(ids) == 0:
                    continue
                yt = (
                    res.results[core][f"ys{t}"].reshape(H, C).astype(np.float32)
                )
                contrib = yt[:, : len(ids)].T * ss[:, None]
                np.add.at(out, ids, contrib)

    return out


# revision 21
# speedup vs baseline: 1.0549x; 1.0549x over previous
"""MoE (T=1024, H=1024, F=2048, E=8, top-k=2) on 8 trn2 cores.

Pair-tensor-parallel expert layout: experts are paired (largest bucket
with smallest) and each pair is served by two cores that split the ffn
dim in half. Core (pair p, half h) holds, for both experts of the pair,
the gate/lin rows [h*F/2,(h+1)*F/2) of w1 and the matching w2 columns,
and processes ALL tokens routed to either expert. fc2 contracts only
the local half of F, so the two halves produce additive partial sums
(b2 is halved on each) that the host scatter-add combines.

Per-core matmul cycles drop from 384*Cmax to 192*(C0+C1), where C0/C1
are the per-slot capacities (max larger-expert count / max smaller),
~4-5% less than pure expert parallelism at identical weight traffic
(12MB bf16 per core, no duplication).

All matmuls are bf16 (weights+activations), fp32 PSUM. A burst of
dependency-free warmup matmuls on a memset tile ramps the PE to full
p-state during the initial DMA window.
"""

import numpy as np
from contextlib import ExitStack

import concourse.bass as bass
import concourse.mybir as mybir
import concourse.tile as tile
from concourse import bacc
from concourse.bass_utils import run_bass_kernel_spmd

import ml_dtypes

T, H, F, E, TOPK = 1024, 1024, 2048, 8, 2
P = 128
FH = F // 2        # per-core half of the ffn inter dim
KH = H // P        # 8   fc1 contraction chunks
MGH = FH // P      # 8   gate m-chunks per expert (lin likewise)
KFH = FH // P      # 8   fc2 contraction chunks (local half)
M2 = H // P        # 8   fc2 output chunks
C_ALIGN = 4
C_MAX = 512
F32 = mybir.dt.float32
BF16 = mybir.dt.bfloat16
NP_BF16 = ml_dtypes.bfloat16

TRACE = False
TRACE_KWARGS = {}
LAST_RESULT = None

_nc_cache = {}


def _build_nc(C0: int, C1: int) -> bass.Bass:
    nc = bacc.Bacc("TRN2", target_bir_lowering=False, debug=False)
    CS = (C0, C1)
    xs0 = nc.dram_tensor("xs0", [P, KH, C0], BF16, kind="ExternalInput")
    xs1 = nc.dram_tensor("xs1", [P, KH, C1], BF16, kind="ExternalInput")
    # per expert slot: [jj, s, g, p, k, n] with m = 2*jj+s (8 gate m-chunks)
    w1s = nc.dram_tensor("w1s", [2, MGH // 2, 2, 2, P, KH, P], BF16, kind="ExternalInput")
    # per expert slot: [mm, s, p, k, n] with m = 2*mm+s (8 out m-chunks, k<8)
    w2s = nc.dram_tensor("w2s", [2, M2 // 2, 2, P, KFH, P], BF16, kind="ExternalInput")
    # per slot: 2*MGH gate+lin bias cols then M2 b2/2 cols
    bs = nc.dram_tensor("bs", [2, P, 2 * MGH + M2], F32, kind="ExternalInput")
    ys0 = nc.dram_tensor("ys0", [M2, P, C0], BF16, kind="ExternalOutput")
    ys1 = nc.dram_tensor("ys1", [M2, P, C1], BF16, kind="ExternalOutput")
    YS = (ys0, ys1)

    silu = mybir.ActivationFunctionType.Silu

    with tile.TileContext(nc) as tc, ExitStack() as ctx:
        consts = ctx.enter_context(tc.tile_pool(name="consts", bufs=1))
        xpool = ctx.enter_context(tc.tile_pool(name="xpool", bufs=2))
        w1pool = ctx.enter_context(tc.tile_pool(name="w1pool", bufs=4))
        w2pool = ctx.enter_context(tc.tile_pool(name="w2pool", bufs=3))
        actpool = ctx.enter_context(tc.tile_pool(name="actpool", bufs=2))
        evpool = ctx.enter_context(tc.tile_pool(name="evpool", bufs=4))
        ypool = ctx.enter_context(tc.tile_pool(name="ypool", bufs=3))
        ps1 = ctx.enter_context(tc.tile_pool(name="ps1", bufs=4, space="PSUM"))
        ps2 = ctx.enter_context(tc.tile_pool(name="ps2", bufs=2, space="PSUM"))
        pswarm = ctx.enter_context(tc.tile_pool(name="pswarm", bufs=1, space="PSUM"))

        # PE p-state warmup during the initial DMA window
        warm = consts.tile([P, 256], BF16)
        nc.gpsimd.memset(warm, 0)
        pwarm = pswarm.tile([P, 256], F32)
        for i in range(24):
            nc.tensor.matmul(
                pwarm, lhsT=warm[:, :P], rhs=warm, start=(i == 0), stop=(i == 23)
            )

        # Startup: slot-0 x and the s=1 w1 slice ride SP ahead of the
        # bulk jj stream; the two s=0 w1 slices + bias + slot-1 x ride ACT
        # in parallel.
        x0_sb = xpool.tile([P, KH, C0], BF16, tag="x0")
        nc.sync.dma_start(out=x0_sb[:, : KH // 2], in_=xs0[:, : KH // 2, :])
        w1_first = w1pool.tile([P, 2, 2, KH, P], BF16, tag="w1")
        nc.scalar.dma_start(out=w1_first[:, 0, 0], in_=w1s[0, 0, 0, 0])
        nc.sync.dma_start(out=x0_sb[:, KH // 2 :], in_=xs0[:, KH // 2 :, :])
        nc.scalar.dma_start(out=w1_first[:, 0, 1], in_=w1s[0, 0, 0, 1])
        nc.sync.dma_start(
            out=w1_first[:, 1], in_=w1s[0, 0, 1].rearrange("g p k n -> p g k n")
        )
        b_sb = consts.tile([P, 2, 2 * MGH + M2], F32)
        nc.scalar.dma_start(out=b_sb[:, 0], in_=bs[0])
        nc.scalar.dma_start(out=b_sb[:, 1], in_=bs[1])
        x1_sb = xpool.tile([P, KH, C1], BF16, tag="x1")
        nc.scalar.dma_start(out=x1_sb, in_=xs1[:, :, :])

        XS = (x0_sb, x1_sb)

        for t in range(2):
            C = CS[t]
            x_sb = XS[t]
            b1_sb = b_sb[:, t, : 2 * MGH]
            b2_sb = b_sb[:, t, 2 * MGH :]
            act_all = actpool.tile([P, KFH, C], BF16, tag=f"act{t}")

            for jj in range(MGH // 2):
                if t == 0 and jj == 0:
                    w1_sb = w1_first
                else:
                    w1_sb = w1pool.tile([P, 2, 2, KH, P], BF16, tag="w1")
                    nc.sync.dma_start(
                        out=w1_sb, in_=w1s[t, jj].rearrange("s g p k n -> p s g k n")
                    )
                for s in range(2):
                    m = 2 * jj + s
                    pg = ps1.tile([P, C], F32, tag="ps1")
                    pl = ps1.tile([P, C], F32, tag="ps1")
                    for k in range(KH):
                        nc.tensor.matmul(
                            pg,
                            lhsT=w1_sb[:, s, 0, k, :],
                            rhs=x_sb[:, k, :],
                            start=(k == 0),
                            stop=(k == KH - 1),
                        )
                    for k in range(KH):
                        nc.tensor.matmul(
                            pl,
                            lhsT=w1_sb[:, s, 1, k, :],
                            rhs=x_sb[:, k, :],
                            start=(k == 0),
                            stop=(k == KH - 1),
                        )
                    gate_sb = evpool.tile([P, C], F32, tag="gate")
                    nc.scalar.activation(gate_sb, pg, silu, bias=b1_sb[:, m : m + 1])
                    lin_sb = evpool.tile([P, C], F32, tag="lin")
                    nc.vector.tensor_scalar_add(
                        lin_sb, pl, b1_sb[:, MGH + m : MGH + m + 1]
                    )
                    nc.vector.tensor_mul(act_all[:, m, :], gate_sb, lin_sb)

            ys = YS[t]
            for mm in range(M2 // 2):
                w2_sb = w2pool.tile([P, 2, KFH, P], BF16, tag="w2")
                if t == 1 and mm == M2 // 2 - 1:
                    nc.sync.dma_start(out=w2_sb[:, 0], in_=w2s[t, mm, 0])
                    nc.sync.dma_start(out=w2_sb[:, 1], in_=w2s[t, mm, 1])
                else:
                    nc.sync.dma_start(
                        out=w2_sb, in_=w2s[t, mm].rearrange("s p k n -> p s k n")
                    )
                y_sb = ypool.tile([P, 2, C], BF16, tag="y")
                last_pair = t == 1 and mm == M2 // 2 - 1
                for s in range(2):
                    m = 2 * mm + s
                    p2 = ps2.tile([P, C], F32, tag="ps2")
                    for k in range(KFH):
                        nc.tensor.matmul(
                            p2,
                            lhsT=w2_sb[:, s, k, :],
                            rhs=act_all[:, k, :],
                            start=(k == 0),
                            stop=(k == KFH - 1),
                        )
                    if last_pair and s == 1:
                        # final slice: split evacuation across both rings so
                        # the first DMA issue overlaps the second bias-add
                        hc = C // 2
                        nc.vector.tensor_scalar_add(
                            y_sb[:, s, :hc], p2[:, :hc], b2_sb[:, m : m + 1]
                        )
                        nc.scalar.dma_start(out=ys[m][:, :hc], in_=y_sb[:, s, :hc])
                        nc.vector.tensor_scalar_add(
                            y_sb[:, s, hc:], p2[:, hc:], b2_sb[:, m : m + 1]
                        )
                        nc.sync.dma_start(out=ys[m][:, hc:], in_=y_sb[:, s, hc:])
                    else:
                        nc.vector.tensor_scalar_add(
                            y_sb[:, s, :], p2, b2_sb[:, m : m + 1]
                        )
                        if last_pair and s == 0:
                            nc.scalar.dma_start(out=ys[m], in_=y_sb[:, s, :])
                if not last_pair:
                    # one merged output DMA per mm-pair (fewer descriptors)
                    nc.scalar.dma_start(
                        out=ys[2 * mm : 2 * mm + 2].rearrange("s p c -> p s c"),
                        in_=y_sb,
                    )

    nc.compile()
    return nc


def _get_nc(C0: int, C1: int) -> bass.Bass:
    key = (C0, C1)
    if key not in _nc_cache:
        _nc_cache[key] = _build_nc(C0, C1)
    return _nc_cache[key]


def _pack_expert_half(w1, b1, w2, b2, e, h):
    """Pack expert e's ffn-half h into device layouts (bf16 weights)."""
    gsl = slice(h * FH, (h + 1) * FH)
    lsl = slice(F + h * FH, F + (h + 1) * FH)
    w1h = np.concatenate([w1[e][gsl], w1[e][lsl]], axis=0)  # [2*FH, H]
    w1c = np.ascontiguousarray(
        w1h.astype(NP_BF16).reshape(2 * MGH, P, KH, P).transpose(0, 3, 2, 1)
    )
    # stack gate/lin m-chunk pairs like the EP kernel: [jj, s, g, p, k, n]
    w1se = np.ascontiguousarray(
        np.stack(
            [
                w1c[:MGH].reshape(MGH // 2, 2, P, KH, P),
                w1c[MGH:].reshape(MGH // 2, 2, P, KH, P),
            ],
            axis=2,
        )
    )
    w2h = w2[e][:, gsl]  # [H, FH]
    w2c = w2h.astype(NP_BF16).reshape(M2, P, KFH, P).transpose(0, 3, 2, 1)
    w2se = np.ascontiguousarray(w2c.reshape(M2 // 2, 2, P, KFH, P))
    b1h = np.concatenate([b1[e][gsl], b1[e][lsl]])  # [2*FH]
    bse = np.ascontiguousarray(
        np.concatenate(
            [b1h.reshape(2 * MGH, P), (0.5 * b2[e]).reshape(M2, P)], 0
        ).T
    )
    return w1se, w2se, bse


def kernel(
    hidden_states,
    token_selected_experts,
    token_final_scales,
    w1,
    b1,
    w2,
    b2,
):
    global LAST_RESULT
    hs = np.ascontiguousarray(np.asarray(hidden_states, dtype=np.float32))
    sel = np.asarray(token_selected_experts, dtype=np.int32)
    scl = np.asarray(token_final_scales, dtype=np.float32)
    w1 = np.asarray(w1, dtype=np.float32)
    b1 = np.asarray(b1, dtype=np.float32)
    w2 = np.asarray(w2, dtype=np.float32)
    b2 = np.asarray(b2, dtype=np.float32)

    nt, hh = hs.shape
    assert (nt, hh) == (T, H), f"unexpected shape {hs.shape}"

    flat_e = sel.reshape(-1)
    slot_tok = np.repeat(np.arange(T, dtype=np.int64), TOPK)
    order = np.argsort(flat_e, kind="stable")
    sorted_tok = slot_tok[order]
    sorted_scl = scl.reshape(-1)[order]
    counts = np.bincount(flat_e, minlength=E)
    starts = np.concatenate([[0], np.cumsum(counts)])

    # pair largest with smallest by bucket size
    rank = np.argsort(counts)[::-1]          # expert ids sorted desc by count
    pairs = [(int(rank[p]), int(rank[E - 1 - p])) for p in range(E // 2)]
    c0_need = max(int(counts[a]) for a, _ in pairs)
    c1_need = max(int(counts[b]) for _, b in pairs)
    C0 = min(C_MAX, -(-max(1, c0_need) // C_ALIGN) * C_ALIGN)
    C1 = min(C_MAX, -(-max(1, c1_need) // C_ALIGN) * C_ALIGN)
    n_chunks = max(1, -(-c0_need // C0), -(-c1_need // C1))

    nc = _get_nc(C0, C1)

    out = np.zeros((T, H), dtype=np.float32)
    for ci in range(n_chunks):
        in_maps = []
        metas = []
        for p, (ea, eb) in enumerate(pairs):
            # gather token batches for both experts of the pair
            xses, idss, scls = [], [], []
            for t, (e, C) in enumerate(((ea, C0), (eb, C1))):
                lo = int(starts[e]) + ci * C
                hi = min(int(starts[e + 1]), lo + C)
                ids = sorted_tok[lo:hi] if hi > lo else np.empty(0, np.int64)
                xg = np.zeros((C, H), dtype=NP_BF16)
                if len(ids):
                    xg[: len(ids)] = hs[ids].astype(NP_BF16)
                xses.append(
                    np.ascontiguousarray(xg.T.reshape(KH, P, C).transpose(1, 0, 2))
                )
                idss.append(ids)
                scls.append(sorted_scl[lo:hi] if len(ids) else None)
            for h in range(2):
                w1a, w2a, ba = _pack_expert_half(w1, b1, w2, b2, ea, h)
                w1b, w2b, bb = _pack_expert_half(w1, b1, w2, b2, eb, h)
                in_maps.append(
                    {
                        "xs0": xses[0],
                        "xs1": xses[1],
                        "w1s": np.ascontiguousarray(np.stack([w1a, w1b])),
                        "w2s": np.ascontiguousarray(np.stack([w2a, w2b])),
                        "bs": np.ascontiguousarray(np.stack([ba, bb])),
                    }
                )
                metas.append((idss, scls))

        res = run_bass_kernel_spmd(
            nc,
            in_maps,
            core_ids=list(range(E)),
            trace=TRACE,
            **TRACE_KWARGS,
        )
        LAST_RESULT = res
        for core in range(E):
            idss, scls = metas[core]
            for t, (ids, ss, C) in enumerate(
                ((idss[0], scls[0], C0), (idss[1], scls[1], C1))
            ):
                if ids is None or len(ids) == 0:
                    continue
                yt = (
                    res.results[core][f"ys{t}"].reshape(H, C).astype(np.float32)
                )
                contrib = yt[:, : len(ids)].T * ss[:, None]
                np.add.at(out, ids, contrib)

    return out
